# revision 1
# baseline (speedup 1.0000x reference)
"""Trainium2 Bass kernel for a 4-layer hierarchical-attention encoder.

Sharding: 8 cores = 2 batch groups x 4 sequence chunks of 512 query tokens.
Each core runs the full layer stack for its 512 tokens; the hidden state is
all-gathered (per batch group) at each layer boundary so every core can
compute full-sequence self-attention K/V locally.

Layouts: activations are kept token-major (TM: [tokens, feat]) for LayerNorm
and feature-major (FM: [feat, tokens]) for matmuls; FM is produced by PE
transposes of the 512-token own chunk plus the all-gather. Matmul operands
are fp16 (fp32 PSUM accumulation); LayerNorm runs in fp32. Softmax skips the
max-subtraction (scores are bounded ~[-2, 2] for this model family) and the
denominator comes from an extra all-ones column appended to V.
"""
import os
import sys

for _p in ("/root/.axon_site/_ro/trn_rl_repo", "/opt/trn_rl_repo", "/opt/pypackages",
           "/root/.axon_site/_ro/pypackages"):
    if os.path.isdir(_p) and _p not in sys.path:
        sys.path.append(_p)

import numpy as np

import concourse.bass as bass
import concourse.mybir as mybir
import concourse.tile as tile
from concourse import bacc
from concourse.bass_utils import run_bass_kernel_spmd

L, E, H, D, F = 4, 512, 8, 64, 2048
B, S, SK = 2, 2048, 1024
NCORES = 8
GROUPS = [[0, 1, 2, 3], [4, 5, 6, 7]]
CH = 512          # tokens per core
ET = E // 128     # 4 feature tiles
TT = CH // 128    # 4 token tiles in own chunk
FT = F // 128     # 16 ffn tiles
KT_SA = S // 128  # 16 key tiles (self)
KT_CA = SK // 128  # 8 key tiles (cross)
HW = 65           # head width incl. denominator column

FP32 = mybir.dt.float32
FP16 = mybir.dt.float16
AF = mybir.ActivationFunctionType
OP = mybir.AluOpType

_CACHE = {}


def _build():
    nc = bacc.Bacc("TRN2", target_bir_lowering=False, debug=False, num_devices=NCORES)

    def din(name, shape, dt=FP16):
        return nc.dram_tensor(name, shape, dt, kind="ExternalInput").ap()

    sen_fm = din("sen_fm", [E, S])            # full batch sequence, feature-major
    own_fm0 = din("own_fm0", [E, CH])         # own chunk, feature-major
    own_tm0 = din("own_tm0", [CH, E])         # own chunk, token-major
    know_fm_d = din("know_fm", [E, SK])
    ident_d = din("ident", [128, 128])
    ones_d = din("ones", [1, 128])

    wq_sa = din("wq_sa", [L, ET, ET, 128, 128])
    wk_sa = din("wk_sa", [L, ET, ET, 128, 128])
    wv_sa = din("wv_sa", [L, ET, 128, H * HW])
    wo_sa = din("wo_sa", [L, ET, 128, E])
    wq_ca = din("wq_ca", [L, ET, ET, 128, 128])
    wk_ca = din("wk_ca", [L, ET, ET, 128, 128])
    wv_ca = din("wv_ca", [L, ET, 128, H * HW])
    wo_ca = din("wo_ca", [L, ET, 128, E])
    w1_d = din("w1", [L, ET, FT, 128, 128])
    w2_d = din("w2", [L, FT, 128, E])

    bq_sa = din("bq_sa", [L, 128, ET], FP32)
    bk_sa = din("bk_sa", [L, 128, ET], FP32)
    bq_ca = din("bq_ca", [L, 128, ET], FP32)
    bk_ca = din("bk_ca", [L, 128, ET], FP32)
    b1_d = din("b1", [L, 128, FT], FP32)
    rbv_sa = din("rbv_sa", [L, 1, H * HW])
    rbo_sa = din("rbo_sa", [L, 1, E])
    rbv_ca = din("rbv_ca", [L, 1, H * HW])
    rbo_ca = din("rbo_ca", [L, 1, E])
    rb2_d = din("rb2", [L, 1, E])
    lng_d = din("lng", [L, 1, E], FP32)
    lnb_d = din("lnb", [L, 1, E], FP32)

    out_d = nc.dram_tensor("out_tm", [CH, E], FP32, kind="ExternalOutput").ap()

    with tile.TileContext(nc) as tc:
        from contextlib import ExitStack
        with ExitStack() as ctx:
            ep = ctx.enter_context
            const_p = ep(tc.tile_pool(name="const", bufs=1))
            know_p = ep(tc.tile_pool(name="know", bufs=4))
            kfm_p = ep(tc.tile_pool(name="kfm", bufs=4))      # [128,2048] SA K fp16
            kca_p = ep(tc.tile_pool(name="kca", bufs=4))      # [128,1024] CA K fp16
            vp_p = ep(tc.tile_pool(name="vp", bufs=27))       # resident V' fp16
            hch_p = ep(tc.tile_pool(name="hch", bufs=6))      # H_fm chunk tiles
            qfm_p = ep(tc.tile_pool(name="qfm", bufs=8))
            attn_p = ep(tc.tile_pool(name="attn", bufs=4))
            ofm_p = ep(tc.tile_pool(name="ofm", bufs=8))      # own_fm
            xfm_p = ep(tc.tile_pool(name="xfm", bufs=5))      # inter_fm / co_fm
            stm_p = ep(tc.tile_pool(name="stm", bufs=8))      # hid/inter/co TM fp16
            out32_p = ep(tc.tile_pool(name="out32", bufs=2))  # final layer fp32 out
            pt_p = ep(tc.tile_pool(name="pt", bufs=6))        # exp(scores^T) fp16
            gel_p = ep(tc.tile_pool(name="gel", bufs=17))
            wl_p = ep(tc.tile_pool(name="wl", bufs=16))       # [128,128] fp16 weights
            wr_p = ep(tc.tile_pool(name="wr", bufs=6))        # [128,<=520] rhs weights
            row_p = ep(tc.tile_pool(name="row", bufs=4))      # [1,<=520] rows
            gb_p = ep(tc.tile_pool(name="gb", bufs=2))        # LN G/B broadcast fp32
            sc_p = ep(tc.tile_pool(name="sc", bufs=3))        # fp32 scratch
            s1_p = ep(tc.tile_pool(name="s1", bufs=2))        # [<=4,512] rows
            st_p = ep(tc.tile_pool(name="st", bufs=8))        # small stats
            ps_p = ep(tc.tile_pool(name="ps", bufs=8, space="PSUM"))
            dram_p = ep(tc.tile_pool(name="dram", bufs=2, space="DRAM"))

            identt = const_p.tile([128, 128], FP16, tag="ident", name="ident")
            nc.sync.dma_start(identt[:], ident_d[:])
            onest = const_p.tile([1, 128], FP16, tag="ones", name="ones")
            nc.sync.dma_start(onest[:], ones_d[:])
            knowfm = []
            for e in range(ET):
                t = know_p.tile([128, SK], FP16, tag="know", name="know")
                nc.sync.dma_start(t[:], know_fm_d[e * 128:(e + 1) * 128, :])
                knowfm.append(t)

            hid = []
            for t in range(TT):
                h = stm_p.tile([128, E], FP16, tag="stm", name="stm")
                nc.sync.dma_start(h[:], own_tm0[t * 128:(t + 1) * 128, :])
                hid.append(h)
            ownfm = []
            for e in range(ET):
                t = ofm_p.tile([128, CH], FP16, tag="ofm", name="ofm")
                nc.sync.dma_start(t[:], own_fm0[e * 128:(e + 1) * 128, :])
                ownfm.append(t)

            def ln_norm(xres, G, Bt, out):
                """out = G*(xres-mean)/(sqrt(bessel_var)+eps) + Bt, rows of 512."""
                stt = st_p.tile([128, 6], FP32, tag="bnst", name="bnst")
                nc.vector.bn_stats(out=stt[:], in_=xres[:])
                mv = st_p.tile([128, 2], FP32, tag="bnmv", name="bnmv")
                nc.vector.bn_aggr(out=mv[:], in_=stt[:])
                sd = st_p.tile([128, 1], FP32, tag="sd", name="sd")
                nc.scalar.activation(sd[:], mv[:, 1:2], AF.Sqrt, scale=float(E) / (E - 1))
                nc.vector.tensor_scalar_add(sd[:], sd[:], 1e-6)
                inv = st_p.tile([128, 1], FP32, tag="inv", name="inv")
                nc.vector.reciprocal_approx_fast(inv[:], sd[:])
                minv = st_p.tile([128, 1], FP32, tag="minv", name="minv")
                nc.vector.tensor_mul(minv[:], mv[:, 0:1], inv[:])
                tmp = sc_p.tile([128, E], FP32, tag="lntmp", name="lntmp")
                nc.vector.tensor_scalar(tmp[:], in0=xres[:], scalar1=inv[:],
                                        scalar2=minv[:], op0=OP.mult, op1=OP.subtract)
                nc.vector.tensor_mul(tmp[:], tmp[:], G[:])
                nc.vector.tensor_add(out[:], tmp[:], Bt[:])

            def transpose_to(dst_tiles, src_tile, t):
                """src [128tok, E] TM tile t -> dst_tiles[e][:, t*128:(t+1)*128]."""
                for e in range(ET):
                    tp = ps_p.tile([128, 128], FP16, tag="ps", name="ps")
                    nc.tensor.transpose(tp[:], src_tile[:, e * 128:(e + 1) * 128],
                                        identt[:])
                    nc.vector.tensor_copy(dst_tiles[e][:, t * 128:(t + 1) * 128], tp[:])

            def load_w16(wdram, l):
                """Load the 16 [128,128] lhsT tiles of one [E,E] weight."""
                ts = {}
                for ei in range(ET):
                    for e in range(ET):
                        wt = wl_p.tile([128, 128], FP16, tag="wl", name="wl")
                        nc.sync.dma_start(wt[:], wdram[l, ei, e])
                        ts[ei, e] = wt
                return ts

            def load_bias(bdram, l, n):
                bt = st_p.tile([128, n], FP32, tag="bias", name="bias", bufs=6)
                nc.sync.dma_start(bt[:], bdram[l])
                return bt

            def kv_proj(kdst, n_tok, src_tiles, src_col0, wk_tiles, bkt):
                """K_fm columns [src_col0:src_col0+n_tok) from FM src tiles."""
                nch = n_tok // 512
                for e in range(ET):
                    for c2 in range(nch):
                        pst = ps_p.tile([128, 512], FP32, tag="ps", name="ps")
                        for ei in range(ET):
                            nc.tensor.matmul(pst[:], wk_tiles[ei, e][:],
                                             src_tiles[ei][:, c2 * 512:(c2 + 1) * 512],
                                             start=(ei == 0), stop=(ei == ET - 1))
                        nc.vector.tensor_scalar_add(
                            kdst[e][:, src_col0 + c2 * 512:src_col0 + (c2 + 1) * 512],
                            pst[:], bkt[:, e:e + 1])

            def v_proj(vdst, kt0, nkt, src_tiles, wv_tiles, rbv):
                """V' token tiles kt0..kt0+nkt-1 (resident fp16)."""
                for ktl in range(nkt):
                    vt = vdst[kt0 + ktl]
                    for half in range(2):
                        pst = ps_p.tile([128, H * HW // 2], FP32, tag="ps", name="ps")
                        cs = half * (H * HW // 2)
                        for ei in range(ET):
                            nc.tensor.matmul(
                                pst[:], src_tiles[ei][:, ktl * 128:(ktl + 1) * 128],
                                wv_tiles[ei][:, cs:cs + H * HW // 2],
                                start=(ei == 0), stop=False)
                        nc.tensor.matmul(pst[:], onest[:], rbv[:, cs:cs + H * HW // 2],
                                         start=False, stop=True)
                        nc.vector.tensor_copy(vt[:, cs:cs + H * HW // 2], pst[:])

            def attention(qfm, kfm, vp, nkt, attn_tiles):
                for hs in range(2):
                    attps = [ps_p.tile([HW, 512], FP32, tag="ps", name="ps")
                             for _ in range(4)]
                    for kt in range(nkt):
                        for h4 in range(4):
                            h = hs * 4 + h4
                            e, r = h // 2, (h % 2) * 64
                            spt = ps_p.tile([128, 512], FP32, tag="ps", name="ps")
                            nc.tensor.matmul(
                                spt[:], kfm[e][r:r + 64, kt * 128:(kt + 1) * 128],
                                qfm[e][r:r + 64, :], start=True, stop=True)
                            pt = pt_p.tile([128, 512], FP16, tag="pt", name="pt")
                            nc.scalar.activation(pt[:], spt[:], AF.Exp, scale=0.125)
                            nc.tensor.matmul(attps[h4][:], vp[kt][:, h * HW:(h + 1) * HW],
                                             pt[:], start=(kt == 0), stop=(kt == nkt - 1))
                    for h4 in range(4):
                        h = hs * 4 + h4
                        e, r = h // 2, (h % 2) * 64
                        # drain PSUM immediately so the next hset's matmuls
                        # get banks; normalize from SBUF afterwards
                        ats = sc_p.tile([64, 512], FP32, tag="ats", name="ats",
                                        bufs=4)
                        nc.scalar.activation(ats[:], attps[h4][0:64, :], AF.Copy)
                        den = s1_p.tile([1, 512], FP32, tag="den", name="den")
                        nc.vector.tensor_copy(den[:], attps[h4][64:65, :])
                        rec = s1_p.tile([1, 512], FP32, tag="rec", name="rec")
                        nc.vector.reciprocal_approx_fast(rec[:], den[:])
                        rb = sc_p.tile([64, 512], FP32, tag="rb", name="rb")
                        nc.gpsimd.partition_broadcast(rb[:], rec[:])
                        nc.vector.tensor_mul(attn_tiles[e][r:r + 64, :],
                                             ats[:], rb[:])

            def out_proj_ln(attn_tiles, wo_tiles, rbo, res_tiles, G, Bt, out_tiles):
                for t in range(TT):
                    pst = ps_p.tile([128, E], FP32, tag="ps", name="ps")
                    for ei in range(ET):
                        nc.tensor.matmul(pst[:], attn_tiles[ei][:, t * 128:(t + 1) * 128],
                                         wo_tiles[ei][:], start=(ei == 0), stop=False)
                    nc.tensor.matmul(pst[:], onest[:], rbo[:], start=False, stop=True)
                    xres = sc_p.tile([128, E], FP32, tag="xres", name="xres")
                    nc.vector.tensor_add(xres[:], pst[:], res_tiles[t][:])
                    ln_norm(xres, G, Bt, out_tiles[t])

            def make_ca_kv(l):
                kca = [kca_p.tile([128, SK], FP16, tag="kca", name="kca")
                       for _ in range(ET)]
                wkt_ca = load_w16(wk_ca, l)
                bkt_ca = load_bias(bk_ca, l, ET)
                kv_proj(kca, SK, knowfm, 0, wkt_ca, bkt_ca)
                vp_ca = [vp_p.tile([128, H * HW], FP16, tag="vp", name="vp")
                         for _ in range(KT_CA)]
                wvt_ca = []
                for ei in range(ET):
                    wt = wr_p.tile([128, H * HW], FP16, tag="wr", name="wr")
                    nc.sync.dma_start(wt[:], wv_ca[l, ei])
                    wvt_ca.append(wt)
                rbv = row_p.tile([1, H * HW], FP16, tag="row", name="row")
                nc.sync.dma_start(rbv[:], rbv_ca[l])
                v_proj(vp_ca, 0, KT_CA, knowfm, wvt_ca, rbv)
                return kca, vp_ca

            ag_out_prev = None
            ca_kv_next = None
            for l in range(L):
                with nc.named_scope(f"L{l}"):
                    if l == 0:
                        kca, vp_ca = make_ca_kv(0)
                    else:
                        kca, vp_ca = ca_kv_next
                    lr = s1_p.tile([1, E], FP32, tag="lnrow", name="lnrow")
                    nc.sync.dma_start(lr[:], lng_d[l])
                    G = gb_p.tile([128, E], FP32, tag="G", name="G")
                    nc.gpsimd.partition_broadcast(G[:], lr[:])
                    lr2 = s1_p.tile([1, E], FP32, tag="lnrow", name="lnrow")
                    nc.sync.dma_start(lr2[:], lnb_d[l])
                    Bt = gb_p.tile([128, E], FP32, tag="B", name="B")
                    nc.gpsimd.partition_broadcast(Bt[:], lr2[:])

                    # ---- SA K/V from the gathered hidden state ----
                    ksa = [kfm_p.tile([128, S], FP16, tag="kfm", name="kfm")
                           for _ in range(ET)]
                    vp_sa = [vp_p.tile([128, H * HW], FP16, tag="vp", name="vp")
                             for _ in range(KT_SA)]
                    wkt_sa = load_w16(wk_sa, l)
                    wvt_sa = []
                    for ei in range(ET):
                        wt = wr_p.tile([128, H * HW], FP16, tag="wr", name="wr")
                        nc.sync.dma_start(wt[:], wv_sa[l, ei])
                        wvt_sa.append(wt)
                    rbvs = row_p.tile([1, H * HW], FP16, tag="row", name="row")
                    nc.sync.dma_start(rbvs[:], rbv_sa[l])
                    bkt_sa = load_bias(bk_sa, l, ET)
                    for ch in range(4):
                        hch = []
                        for ei in range(ET):
                            ht = hch_p.tile([128, 512], FP16, tag="hch", name="hch")
                            if l == 0:
                                nc.sync.dma_start(
                                    ht[:], sen_fm[ei * 128:(ei + 1) * 128,
                                                  ch * 512:(ch + 1) * 512])
                            else:
                                nc.sync.dma_start(
                                    ht[:], ag_out_prev[ch * 512 + ei * 128:
                                                       ch * 512 + (ei + 1) * 128, :])
                            hch.append(ht)
                        kv_proj(ksa, 512, hch, ch * 512, wkt_sa, bkt_sa)
                        v_proj(vp_sa, ch * 4, 4, hch, wvt_sa, rbvs)

                    # ---- SA Q from own chunk (l>0: computed during prev AG) ----
                    if l == 0:
                        qsa = [qfm_p.tile([128, 512], FP16, tag="qfm", name="qfm")
                               for _ in range(ET)]
                        wqt_sa = load_w16(wq_sa, l)
                        bqt = load_bias(bq_sa, l, ET)
                        for e in range(ET):
                            pst = ps_p.tile([128, 512], FP32, tag="ps", name="ps")
                            for ei in range(ET):
                                nc.tensor.matmul(pst[:], wqt_sa[ei, e][:], ownfm[ei][:],
                                                 start=(ei == 0), stop=(ei == ET - 1))
                            nc.vector.tensor_scalar_add(qsa[e][:], pst[:],
                                                        bqt[:, e:e + 1])
                    else:
                        qsa = qsa_next

                    # ---- SA attention + out-proj + LN1 ----
                    attn = [attn_p.tile([128, 512], FP16, tag="attn", name="attn")
                            for _ in range(ET)]
                    attention(qsa, ksa, vp_sa, KT_SA, attn)
                    wot = []
                    for ei in range(ET):
                        wt = wr_p.tile([128, E], FP16, tag="wr", name="wr")
                        nc.sync.dma_start(wt[:], wo_sa[l, ei])
                        wot.append(wt)
                    rbo = row_p.tile([1, E], FP16, tag="row", name="row")
                    nc.sync.dma_start(rbo[:], rbo_sa[l])
                    inter = [stm_p.tile([128, E], FP16, tag="stm", name="stm")
                             for _ in range(TT)]
                    out_proj_ln(attn, wot, rbo, hid, G, Bt, inter)

                    interfm = [xfm_p.tile([128, CH], FP16, tag="xfm", name="xfm")
                               for _ in range(ET)]
                    for t in range(TT):
                        transpose_to(interfm, inter[t], t)

                    # ---- CA Q + attention + out-proj + LN2 ----
                    qca = [qfm_p.tile([128, 512], FP16, tag="qfm", name="qfm")
                           for _ in range(ET)]
                    wqt_ca = load_w16(wq_ca, l)
                    bqt_ca = load_bias(bq_ca, l, ET)
                    for e in range(ET):
                        pst = ps_p.tile([128, 512], FP32, tag="ps", name="ps")
                        for ei in range(ET):
                            nc.tensor.matmul(pst[:], wqt_ca[ei, e][:], interfm[ei][:],
                                             start=(ei == 0), stop=(ei == ET - 1))
                        nc.vector.tensor_scalar_add(qca[e][:], pst[:],
                                                    bqt_ca[:, e:e + 1])

                    attn2 = [attn_p.tile([128, 512], FP16, tag="attn", name="attn")
                             for _ in range(ET)]
                    attention(qca, kca, vp_ca, KT_CA, attn2)
                    wot2 = []
                    for ei in range(ET):
                        wt = wr_p.tile([128, E], FP16, tag="wr", name="wr")
                        nc.sync.dma_start(wt[:], wo_ca[l, ei])
                        wot2.append(wt)
                    rbo2 = row_p.tile([1, E], FP16, tag="row", name="row")
                    nc.sync.dma_start(rbo2[:], rbo_ca[l])
                    co = [stm_p.tile([128, E], FP16, tag="stm", name="stm")
                          for _ in range(TT)]
                    out_proj_ln(attn2, wot2, rbo2, inter, G, Bt, co)

                    cofm = [xfm_p.tile([128, CH], FP16, tag="xfm", name="xfm")
                            for _ in range(ET)]
                    for t in range(TT):
                        transpose_to(cofm, co[t], t)

                    # ---- FFN: h1 for all ft (gelu resident), then h2 per t ----
                    rb2 = row_p.tile([1, E], FP16, tag="row", name="row")
                    nc.sync.dma_start(rb2[:], rb2_d[l])
                    b1t = load_bias(b1_d, l, FT)
                    gel = []
                    for ft in range(FT):
                        pst = ps_p.tile([128, 512], FP32, tag="ps", name="ps")
                        for ei in range(ET):
                            wt = wl_p.tile([128, 128], FP16, tag="wl", name="wl")
                            nc.sync.dma_start(wt[:], w1_d[l, ei, ft])
                            nc.tensor.matmul(pst[:], wt[:], cofm[ei][:],
                                             start=(ei == 0), stop=(ei == ET - 1))
                        gt = gel_p.tile([128, 512], FP16, tag="gel", name="gel")
                        nc.scalar.activation(gt[:], pst[:], AF.Gelu,
                                             bias=b1t[:, ft:ft + 1])
                        gel.append(gt)
                    w2ts = []
                    for ft in range(FT):
                        w2t = wr_p.tile([128, E], FP16, tag="w2r", name="w2r", bufs=17)
                        nc.sync.dma_start(w2t[:], w2_d[l, ft])
                        w2ts.append(w2t)
                    h2ps = [ps_p.tile([128, E], FP32, tag="ps", name="ps")
                            for _ in range(TT)]
                    for t in range(TT):
                        for ft in range(FT):
                            nc.tensor.matmul(h2ps[t][:], gel[ft][:, t * 128:(t + 1) * 128],
                                             w2ts[ft][:], start=(ft == 0), stop=False)
                    if l == L - 1:
                        hidn = [out32_p.tile([128, E], FP32, tag="out32", name="out32")
                                for _ in range(TT)]
                    else:
                        hidn = [stm_p.tile([128, E], FP16, tag="stm", name="stm")
                                for _ in range(TT)]
                    for t in range(TT):
                        nc.tensor.matmul(h2ps[t][:], onest[:], rb2[:],
                                         start=False, stop=True)
                        xres = sc_p.tile([128, E], FP32, tag="xres", name="xres")
                        nc.vector.tensor_add(xres[:], h2ps[t][:], co[t][:])
                        ln_norm(xres, G, Bt, hidn[t])
                        if l == L - 1:
                            nc.sync.dma_start(out_d[t * 128:(t + 1) * 128, :], hidn[t][:])

                    if l < L - 1:
                        ownfm_n = [ofm_p.tile([128, CH], FP16, tag="ofm", name="ofm")
                                   for _ in range(ET)]
                        for t in range(TT):
                            transpose_to(ownfm_n, hidn[t], t)
                        ag_in = dram_p.tile([CH, E], FP16, tag="agin", name="agin")
                        for e in range(ET):
                            nc.sync.dma_start(ag_in[e * 128:(e + 1) * 128, :],
                                              ownfm_n[e][:])
                        ag_out = dram_p.tile([S, E], FP16, tag="agout", name="agout")
                        nc.gpsimd.collective_compute(
                            "AllGather", OP.bypass, replica_groups=GROUPS,
                            ins=[ag_in.opt()], outs=[ag_out.opt()])
                        # AG-independent work for the next layer fills the
                        # collective latency: CA K/V from know + Q from own chunk.
                        ca_kv_next = make_ca_kv(l + 1)
                        qsa_next = [qfm_p.tile([128, 512], FP16, tag="qfm",
                                               name="qfm") for _ in range(ET)]
                        wqt_n = load_w16(wq_sa, l + 1)
                        bqt_n = load_bias(bq_sa, l + 1, ET)
                        for e in range(ET):
                            pst = ps_p.tile([128, 512], FP32, tag="ps", name="ps")
                            for ei in range(ET):
                                nc.tensor.matmul(pst[:], wqt_n[ei, e][:], ownfm_n[ei][:],
                                                 start=(ei == 0), stop=(ei == ET - 1))
                            nc.vector.tensor_scalar_add(qsa_next[e][:], pst[:],
                                                        bqt_n[:, e:e + 1])
                        ag_out_prev = ag_out
                        ownfm = ownfm_n
                        hid = hidn

    nc.compile()
    return nc


def _prep_inputs(sen, know, sa_qkv_w, sa_qkv_b, sa_out_w, sa_out_b,
                 ca_qkv_w, ca_qkv_b, ca_out_w, ca_out_b,
                 ff_w1, ff_b1, ff_w2, ff_b2, ln_g, ln_b):
    """Host-side weight packing shared by all cores + per-core activations."""
    f16, f32 = np.float16, np.float32

    def tile4(w):  # [L,E,E] -> [L,ET,ET,128,128] (ein, eout)
        return np.ascontiguousarray(
            w.reshape(L, ET, 128, ET, 128).transpose(0, 1, 3, 2, 4).astype(f16))

    def padv(w, b):  # [L,E,E],[L,E] -> [L,ET,128,H*HW], [L,1,H*HW]
        wp = np.zeros((L, E, H, HW), f32)
        wp[:, :, :, :D] = w.reshape(L, E, H, D)
        bp = np.zeros((L, H, HW), f32)
        bp[:, :, :D] = b.reshape(L, H, D)
        bp[:, :, D] = 1.0
        return (np.ascontiguousarray(wp.reshape(L, ET, 128, H * HW).astype(f16)),
                np.ascontiguousarray(bp.reshape(L, 1, H * HW).astype(f16)))

    wv_sa_p, rbv_sa_h = padv(sa_qkv_w[:, 2], sa_qkv_b[:, 2])
    wv_ca_p, rbv_ca_h = padv(ca_qkv_w[:, 2], ca_qkv_b[:, 2])

    common = {
        "ident": np.eye(128, dtype=f16),
        "ones": np.ones((1, 128), f16),
        "wq_sa": tile4(sa_qkv_w[:, 0]), "wk_sa": tile4(sa_qkv_w[:, 1]),
        "wv_sa": wv_sa_p,
        "wo_sa": np.ascontiguousarray(sa_out_w.reshape(L, ET, 128, E).astype(f16)),
        "wq_ca": tile4(ca_qkv_w[:, 0]), "wk_ca": tile4(ca_qkv_w[:, 1]),
        "wv_ca": wv_ca_p,
        "wo_ca": np.ascontiguousarray(ca_out_w.reshape(L, ET, 128, E).astype(f16)),
        "w1": np.ascontiguousarray(
            ff_w1.reshape(L, ET, 128, FT, 128).transpose(0, 1, 3, 2, 4).astype(f16)),
        "w2": np.ascontiguousarray(ff_w2.reshape(L, FT, 128, E).astype(f16)),
        "bq_sa": np.ascontiguousarray(
            sa_qkv_b[:, 0].reshape(L, ET, 128).transpose(0, 2, 1)),
        "bk_sa": np.ascontiguousarray(
            sa_qkv_b[:, 1].reshape(L, ET, 128).transpose(0, 2, 1)),
        "bq_ca": np.ascontiguousarray(
            ca_qkv_b[:, 0].reshape(L, ET, 128).transpose(0, 2, 1)),
        "bk_ca": np.ascontiguousarray(
            ca_qkv_b[:, 1].reshape(L, ET, 128).transpose(0, 2, 1)),
        "b1": np.ascontiguousarray(
            ff_b1.reshape(L, FT, 128).transpose(0, 2, 1)),
        "rbv_sa": rbv_sa_h, "rbv_ca": rbv_ca_h,
        "rbo_sa": np.ascontiguousarray(sa_out_b[:, None, :].astype(f16)),
        "rbo_ca": np.ascontiguousarray(ca_out_b[:, None, :].astype(f16)),
        "rb2": np.ascontiguousarray(ff_b2[:, None, :].astype(f16)),
        "lng": np.ascontiguousarray(ln_g[:, None, :]),
        "lnb": np.ascontiguousarray(ln_b[:, None, :]),
    }
    in_maps = []
    for core in range(NCORES):
        g, c = core // 4, core % 4
        m = dict(common)
        m["sen_fm"] = np.ascontiguousarray(sen[g].T.astype(f16))
        m["own_fm0"] = np.ascontiguousarray(sen[g, c * CH:(c + 1) * CH].T.astype(f16))
        m["own_tm0"] = np.ascontiguousarray(sen[g, c * CH:(c + 1) * CH].astype(f16))
        m["know_fm"] = np.ascontiguousarray(know[g].T.astype(f16))
        in_maps.append(m)
    return in_maps


def kernel(**inputs):
    inputs = {k: np.asarray(v, dtype=np.float32) for k, v in inputs.items()}
    if "nc" not in _CACHE:
        _CACHE["nc"] = _build()
    nc = _CACHE["nc"]
    in_maps = _prep_inputs(**inputs)
    res = run_bass_kernel_spmd(nc, in_maps, list(range(NCORES)))
    out = np.empty((B, S, E), np.float32)
    for core in range(NCORES):
        g, c = core // 4, core % 4
        out[g, c * CH:(c + 1) * CH] = res.results[core]["out_tm"]
    return out



# revision 11
# speedup vs baseline: 1.3682x; 1.3682x over previous
"""Trainium2 Bass kernel for a 4-layer hierarchical-attention encoder.

Sharding: 8 cores = 2 batch groups x 4 sequence chunks of 512 query tokens.
Each core runs the full layer stack for its 512 tokens; the hidden state is
AllGathered (feature-major, 512KB) at each layer boundary and every core
recomputes full-sequence self-attention K/V locally -- the dense projection
burst also keeps the PE at its top p-state.  Cross-attention K/V from the
static `know` are computed locally per layer during the AllGather window.

Attention is software-pipelined (scores for tile kt issue while PV matmuls
for kt-2 retire) so the PE does not sit behind the softmax exp; exp tiles
are split between the Activation engine (exact exp) and the Vector engine
(Schraudolph fp16 bit-trick exp; the ~3% per-prob error washes out in the
softmax normalization and LayerNorm).  Weights load as whole-layer slabs.
"""
import os
import sys

for _p in ("/root/.axon_site/_ro/trn_rl_repo", "/opt/trn_rl_repo", "/opt/pypackages",
           "/root/.axon_site/_ro/pypackages"):
    if os.path.isdir(_p) and _p not in sys.path:
        sys.path.append(_p)

import numpy as np

import concourse.bass as bass
import concourse.mybir as mybir
import concourse.tile as tile
from concourse import bacc
from concourse.bass_utils import run_bass_kernel_spmd

L, E, H, D, F = 4, 512, 8, 64, 2048
B, S, SK = 2, 2048, 1024
NCORES = 8
GROUPS = [[0, 1, 2, 3], [4, 5, 6, 7]]
CH = 512          # query tokens per core
ET = E // 128     # 4 feature tiles
TT = CH // 128    # 4 token tiles in own chunk
FT = F // 128     # 16 ffn tiles
KT_SA = S // 128  # 16 key tiles (self)
KT_CA = SK // 128  # 8 key tiles (cross)
HW = 65           # head width incl. denominator column
HHW = H * HW      # 520

FP32 = mybir.dt.float32
FP16 = mybir.dt.float16
I16 = mybir.dt.int16
AF = mybir.ActivationFunctionType
OP = mybir.AluOpType

# Schraudolph fp16 exp: e^(0.125*x) ~= bitcast_fp16(int16(round(A*x + B)))
SEXP_A = float(1024.0 / np.log(2.0)) * 0.125
SEXP_B = 15301.0
USE_SEXP = os.environ.get("USE_SEXP", "1") == "1"

_CACHE = {}


def _build():
    nc = bacc.Bacc("TRN2", target_bir_lowering=False, debug=False, num_devices=NCORES)

    def din(name, shape, dt=FP16):
        return nc.dram_tensor(name, shape, dt, kind="ExternalInput").ap()

    sen_fm = din("sen_fm", [E, S])             # full batch-group seq, feature-major
    own_fm0 = din("own_fm0", [128, ET * CH])   # own chunk, feature-major packed
    own_tm0 = din("own_tm0", [CH, E])          # own chunk, token-major
    know_fm_d = din("know_fm", [E, SK])        # full know, feature-major
    ident_d = din("ident", [128, 128])
    ones_d = din("ones", [1, 128])

    wq_sa_d = din("wq_sa", [L, 128, ET * ET * 128])
    wk_sa_d = din("wk_sa", [L, 128, ET * ET * 128])
    wv_sa_d = din("wv_sa", [L, 128, ET * HHW])
    wo_sa_d = din("wo_sa", [L, 128, ET * E])
    wq_ca_d = din("wq_ca", [L, 128, ET * ET * 128])
    wk_ca_d = din("wk_ca", [L, 128, ET * ET * 128])
    wv_ca_d = din("wv_ca", [L, 128, ET * HHW])
    wo_ca_d = din("wo_ca", [L, 128, ET * E])
    w1_d = din("w1", [L, 4, 128, 4 * ET * 128])   # quarter-slabs, ft-major
    w2_d = din("w2", [L, 4, 128, 4 * E])          # quarter-slabs, ft-major

    bq_sa_d = din("bq_sa", [L, 128, ET], FP32)
    bk_sa_d = din("bk_sa", [L, 128, ET], FP32)
    bq_ca_d = din("bq_ca", [L, 128, ET], FP32)
    bk_ca_d = din("bk_ca", [L, 128, ET], FP32)
    b1_d = din("b1", [L, 128, FT], FP32)
    rbv_sa_d = din("rbv_sa", [L, 1, HHW])
    rbo_sa_d = din("rbo_sa", [L, 1, E])
    rbv_ca_d = din("rbv_ca", [L, 1, HHW])
    rbo_ca_d = din("rbo_ca", [L, 1, E])
    rb2_d = din("rb2", [L, 1, E])
    lng_d = din("lng", [L, 1, E], FP32)
    lnb_d = din("lnb", [L, 1, E], FP32)

    out_d = nc.dram_tensor("out_tm", [CH, E], FP32, kind="ExternalOutput").ap()

    with tile.TileContext(nc) as tc:
        from contextlib import ExitStack
        with ExitStack() as ctx:
            ep = ctx.enter_context
            const_p = ep(tc.tile_pool(name="const", bufs=1))
            know_p = ep(tc.tile_pool(name="know", bufs=4))
            hch_p = ep(tc.tile_pool(name="hch", bufs=8))      # [128,512] hidden fm
            kfm_p = ep(tc.tile_pool(name="kfm", bufs=4))      # [128,2048] SA K fp16
            vsa_p = ep(tc.tile_pool(name="vsa", bufs=4))      # [128,2080] SA V fp16
            kca_p = ep(tc.tile_pool(name="kca", bufs=4))      # [128,1024] CA K fp16
            vca_p = ep(tc.tile_pool(name="vca", bufs=4))      # [128,1040] CA V fp16
            xfm_p = ep(tc.tile_pool(name="xfm", bufs=4))      # [128,2048] fm acts
            qfm_p = ep(tc.tile_pool(name="qfm", bufs=4))
            attn_p = ep(tc.tile_pool(name="attn", bufs=8))
            stm_p = ep(tc.tile_pool(name="stm", bufs=10))     # hid/inter/co TM fp16
            out32_p = ep(tc.tile_pool(name="out32", bufs=4))
            pt_p = ep(tc.tile_pool(name="pt", bufs=4))        # exp(scores) rings
            gel_p = ep(tc.tile_pool(name="gel", bufs=3))
            wsl_p = ep(tc.tile_pool(name="wsl", bufs=1))      # weight slabs
            row_p = ep(tc.tile_pool(name="row", bufs=4))      # [1,<=520] rows
            gb_p = ep(tc.tile_pool(name="gb", bufs=1))        # LN G/B broadcast fp32
            sc_p = ep(tc.tile_pool(name="sc", bufs=2))        # fp32 scratch
            s1_p = ep(tc.tile_pool(name="s1", bufs=1))        # [<=4,512] rows
            st_p = ep(tc.tile_pool(name="st", bufs=8))        # small stats
            ps_p = ep(tc.tile_pool(name="ps", bufs=6, space="PSUM"))
            dram_p = ep(tc.tile_pool(name="dram", bufs=2, space="DRAM"))

            identt = const_p.tile([128, 128], FP16, tag="ident", name="ident")
            nc.sync.dma_start(identt[:], ident_d[:])
            onest = const_p.tile([1, 128], FP16, tag="ones", name="ones")
            nc.sync.dma_start(onest[:], ones_d[:])
            knowfm = []
            for e in range(ET):
                t = know_p.tile([128, SK], FP16, tag="know", name="know")
                nc.sync.dma_start(t[:], know_fm_d[e * 128:(e + 1) * 128, :])
                knowfm.append(t)
            ownfm = xfm_p.tile([128, ET * CH], FP16, tag="xfm", name="ownfm")
            nc.sync.dma_start(ownfm[:], own_fm0[:])
            hid = []
            for t in range(TT):
                h = stm_p.tile([128, E], FP16, tag="stm", name="hid")
                nc.sync.dma_start(h[:], own_tm0[t * 128:(t + 1) * 128, :])
                hid.append(h)

            # ---------------- weight loaders (single-buffered slabs) ----
            def slab(dram, l, cols, tg, bufs=1):
                t = wsl_p.tile([128, cols], FP16, tag=tg, name=tg, bufs=bufs)
                nc.sync.dma_start(t[:], dram[l])
                return t

            def srow(dram, l, cols, tg="row"):
                t = row_p.tile([1, cols], FP16, tag=tg, name=tg, bufs=8)
                nc.sync.dma_start(t[:], dram[l])
                return t

            def sbias(dram, l, cols, tg="bias"):
                t = st_p.tile([128, cols], FP32, tag=tg, name=tg, bufs=8)
                nc.sync.dma_start(t[:], dram[l])
                return t

            def load_sa_kvq(l):
                return dict(
                    wq=slab(wq_sa_d, l, ET * ET * 128, "wq_sa"),
                    wk=slab(wk_sa_d, l, ET * ET * 128, "wk_sa"),
                    wv=slab(wv_sa_d, l, ET * HHW, "wv_sa"),
                    bq=sbias(bq_sa_d, l, ET), bk=sbias(bk_sa_d, l, ET),
                    rbv=srow(rbv_sa_d, l, HHW))

            def load_sa_o(l):
                return dict(wo=slab(wo_sa_d, l, ET * E, "wo_sa"),
                            rbo=srow(rbo_sa_d, l, E))

            def load_ca_kv(l):
                return dict(
                    wk=slab(wk_ca_d, l, ET * ET * 128, "wk_ca"),
                    wv=slab(wv_ca_d, l, ET * HHW, "wv_ca"),
                    bk=sbias(bk_ca_d, l, ET), rbv=srow(rbv_ca_d, l, HHW))

            def load_ca_qo(l):
                return dict(
                    wq=slab(wq_ca_d, l, ET * ET * 128, "wq_ca"),
                    wo=slab(wo_ca_d, l, ET * E, "wo_ca"),
                    bq=sbias(bq_ca_d, l, ET), rbo=srow(rbo_ca_d, l, E))

            def load_ffn_w(l):
                return dict(
                    w1=[slab(w1_d[l], q, 4 * ET * 128, "w1q", bufs=2)
                        for q in range(4)],
                    w2=[slab(w2_d[l], q, 4 * E, "w2q", bufs=2) for q in range(4)],
                    b1=sbias(b1_d, l, FT), rb2=srow(rb2_d, l, E))

            def load_ln(l):
                lr = s1_p.tile([1, E], FP32, tag="lnrow", name="lnrow")
                nc.sync.dma_start(lr[:], lng_d[l])
                G = gb_p.tile([128, E], FP32, tag="G", name="G")
                nc.gpsimd.partition_broadcast(G[:], lr[:])
                lr2 = s1_p.tile([1, E], FP32, tag="B", name="lnrow2")
                nc.sync.dma_start(lr2[:], lnb_d[l])
                Bt = gb_p.tile([128, E], FP32, tag="Bb", name="Bb")
                nc.gpsimd.partition_broadcast(Bt[:], lr2[:])
                return G, Bt

            # ---------------- compute helpers ----------------
            def ln_norm(xres, G, Bt, out):
                """out = G*(xres-mean)/(sqrt(bessel_var)+eps) + Bt."""
                stt = st_p.tile([128, 6], FP32, tag="bnst", name="bnst")
                nc.vector.bn_stats(out=stt[:], in_=xres[:])
                mv = st_p.tile([128, 2], FP32, tag="bnmv", name="bnmv")
                nc.vector.bn_aggr(out=mv[:], in_=stt[:])
                sd = st_p.tile([128, 1], FP32, tag="sd", name="sd")
                nc.scalar.activation(sd[:], mv[:, 1:2], AF.Sqrt, scale=float(E) / (E - 1))
                nc.vector.tensor_scalar_add(sd[:], sd[:], 1e-6)
                inv = st_p.tile([128, 1], FP32, tag="inv", name="inv")
                nc.vector.reciprocal_approx_fast(inv[:], sd[:])
                minv = st_p.tile([128, 1], FP32, tag="minv", name="minv")
                nc.vector.tensor_mul(minv[:], mv[:, 0:1], inv[:])
                tmp = sc_p.tile([128, E], FP32, tag="lntmp", name="lntmp")
                nc.vector.tensor_scalar(tmp[:], in0=xres[:], scalar1=inv[:],
                                        scalar2=minv[:], op0=OP.mult, op1=OP.subtract)
                nc.vector.tensor_mul(tmp[:], tmp[:], G[:])
                nc.vector.tensor_add(out[:], tmp[:], Bt[:])

            def transpose_to(dst, src_tile, t):
                """src [128tok, E] TM tile t -> dst[:, e*CH + t*128 ...] (fm)."""
                for e in range(ET):
                    tp = ps_p.tile([128, 128], FP16, tag="ps", name="ps")
                    nc.tensor.transpose(tp[:], src_tile[:, e * 128:(e + 1) * 128],
                                        identt[:])
                    nc.vector.tensor_copy(dst[:, e * CH + t * 128:e * CH + (t + 1) * 128],
                                          tp[:])

            def q_proj(src_fm, wq, bq):
                qs = []
                for e in range(ET):
                    pst = ps_p.tile([128, CH], FP32, tag="ps", name="ps")
                    for ei in range(ET):
                        nc.tensor.matmul(
                            pst[:], wq[:, (ei * ET + e) * 128:(ei * ET + e + 1) * 128],
                            src_fm[:, ei * CH:(ei + 1) * CH],
                            start=(ei == 0), stop=(ei == ET - 1))
                    qt = qfm_p.tile([128, CH], FP16, tag="qfm", name="qfm")
                    nc.vector.tensor_scalar_add(qt[:], pst[:], bq[:, e:e + 1])
                    qs.append(qt)
                return qs

            def kv_all(w, src_fn):
                """Full-sequence SA K/V from per-chunk hidden fm tiles.

                src_fn(ch) -> list of 4 [128,512] fm tiles for chunk ch."""
                kfm = [kfm_p.tile([128, S], FP16, tag="kfm", name="kfm")
                       for _ in range(ET)]
                vsa = [vsa_p.tile([128, 4 * HHW], FP16, tag="vsa", name="vsa")
                       for _ in range(4)]
                for ch in range(4):
                    hch = src_fn(ch)
                    for e in range(ET):
                        pst = ps_p.tile([128, CH], FP32, tag="ps", name="ps")
                        for ei in range(ET):
                            nc.tensor.matmul(
                                pst[:],
                                w["wk"][:, (ei * ET + e) * 128:(ei * ET + e + 1) * 128],
                                hch[ei][:], start=(ei == 0), stop=(ei == ET - 1))
                        nc.vector.tensor_scalar_add(
                            kfm[e][:, ch * CH:(ch + 1) * CH], pst[:], w["bk"][:, e:e + 1])
                    for lt in range(TT):
                        for half in range(2):
                            cs = half * (HHW // 2)
                            pst = ps_p.tile([128, HHW // 2], FP32, tag="ps", name="ps")
                            for ei in range(ET):
                                nc.tensor.matmul(
                                    pst[:], hch[ei][:, lt * 128:(lt + 1) * 128],
                                    w["wv"][:, ei * HHW + cs:ei * HHW + cs + HHW // 2],
                                    start=(ei == 0), stop=False)
                            nc.tensor.matmul(pst[:], onest[:],
                                             w["rbv"][:, cs:cs + HHW // 2],
                                             start=False, stop=True)
                            nc.scalar.activation(
                                vsa[ch][:, lt * HHW + cs:lt * HHW + cs + HHW // 2],
                                pst[:], AF.Copy)
                return kfm, vsa

            def make_ca_kv(w):
                """Full CA K/V from resident know (collective-window filler)."""
                kca = [kca_p.tile([128, SK], FP16, tag="kca", name="kca")
                       for _ in range(ET)]
                vca = [vca_p.tile([128, 2 * HHW], FP16, tag="vca", name="vca")
                       for _ in range(4)]
                for e in range(ET):
                    for cc in range(2):
                        pst = ps_p.tile([128, CH], FP32, tag="ps", name="ps")
                        for ei in range(ET):
                            nc.tensor.matmul(
                                pst[:],
                                w["wk"][:, (ei * ET + e) * 128:(ei * ET + e + 1) * 128],
                                knowfm[ei][:, cc * CH:(cc + 1) * CH],
                                start=(ei == 0), stop=(ei == ET - 1))
                        nc.vector.tensor_scalar_add(
                            kca[e][:, cc * CH:(cc + 1) * CH], pst[:], w["bk"][:, e:e + 1])
                for kt in range(KT_CA):
                    for half in range(2):
                        cs = half * (HHW // 2)
                        pst = ps_p.tile([128, HHW // 2], FP32, tag="ps", name="ps")
                        for ei in range(ET):
                            nc.tensor.matmul(
                                pst[:], knowfm[ei][:, kt * 128:(kt + 1) * 128],
                                w["wv"][:, ei * HHW + cs:ei * HHW + cs + HHW // 2],
                                start=(ei == 0), stop=False)
                        nc.tensor.matmul(pst[:], onest[:], w["rbv"][:, cs:cs + HHW // 2],
                                         start=False, stop=True)
                        nc.scalar.activation(
                            vca[kt // 2][:, (kt % 2) * HHW + cs:(kt % 2) * HHW + cs + HHW // 2],
                            pst[:], AF.Copy)
                return kca, vca

            def attention(qfm, kfm, vp_at, nkt, attn_tiles):
                PD = 2  # exp pipeline distance in kt tiles
                for hp in range(ET):
                    attps = [ps_p.tile([HW, CH], FP32, tag="attps", name="attps",
                                       bufs=2)
                             for _ in range(2)]
                    pts = {}

                    def scores(kt):
                        for j in (0, 1):
                            spt = ps_p.tile([128, CH], FP32, tag="ps", name="spt")
                            nc.tensor.matmul(
                                spt[:],
                                kfm[hp][j * 64:(j + 1) * 64, kt * 128:(kt + 1) * 128],
                                qfm[hp][j * 64:(j + 1) * 64, :], start=True, stop=True)
                            if j == 0 or not USE_SEXP:
                                pt = pt_p.tile([128, CH], FP16, tag="pte", name="pte")
                                nc.scalar.activation(pt[:], spt[:], AF.Exp, scale=0.125)
                                pts[kt, j] = pt[:]
                            else:
                                pti = pt_p.tile([128, CH], I16, tag="ptv", name="ptv")
                                nc.vector.tensor_scalar(pti[:], in0=spt[:],
                                                        scalar1=SEXP_A, scalar2=SEXP_B,
                                                        op0=OP.mult, op1=OP.add)
                                pts[kt, j] = pti[:].bitcast(FP16)

                    def pv(kt):
                        for j in (0, 1):
                            h = hp * 2 + j
                            vtile, col0 = vp_at(kt, h)
                            nc.tensor.matmul(attps[j][:], vtile[:, col0:col0 + HW],
                                             pts.pop((kt, j)),
                                             start=(kt == 0), stop=(kt == nkt - 1))

                    for kt in range(nkt):
                        scores(kt)
                        if kt >= PD:
                            pv(kt - PD)
                    for kt in range(nkt - PD, nkt):
                        pv(kt)
                    for j in (0, 1):
                        den = s1_p.tile([1, CH], FP32, tag="den", name="den")
                        nc.scalar.activation(den[:], attps[j][64:65, :], AF.Copy)
                        rec = s1_p.tile([1, CH], FP32, tag="rec", name="rec")
                        nc.vector.reciprocal_approx_fast(rec[:], den[:])
                        rb = sc_p.tile([64, CH], FP32, tag="rb", name="rb")
                        nc.gpsimd.partition_broadcast(rb[:], rec[:])
                        nc.vector.tensor_mul(attn_tiles[hp][j * 64:(j + 1) * 64, :],
                                             attps[j][0:64, :], rb[:])

            def out_proj_ln(attn_tiles, w, res, G, Bt, out_tiles):
                for t in range(TT):
                    pst = ps_p.tile([128, E], FP32, tag="ps", name="ps")
                    for ei in range(ET):
                        nc.tensor.matmul(pst[:], attn_tiles[ei][:, t * 128:(t + 1) * 128],
                                         w["wo"][:, ei * E:(ei + 1) * E],
                                         start=(ei == 0), stop=False)
                    nc.tensor.matmul(pst[:], onest[:], w["rbo"][:], start=False, stop=True)
                    xres = sc_p.tile([128, E], FP32, tag="xres", name="xres")
                    nc.vector.tensor_add(xres[:], pst[:], res[t][:])
                    ln_norm(xres, G, Bt, out_tiles[t])

            def hch_from_sen(ch):
                tiles = []
                for ei in range(ET):
                    t = hch_p.tile([128, CH], FP16, tag="hch", name="hch")
                    nc.sync.dma_start(t[:], sen_fm[ei * 128:(ei + 1) * 128,
                                                   ch * CH:(ch + 1) * CH])
                    tiles.append(t)
                return tiles

            def hch_from_ag(ag_out, ch):
                tiles = []
                for ei in range(ET):
                    t = hch_p.tile([128, CH], FP16, tag="hch", name="hch")
                    nc.sync.dma_start(
                        t[:], ag_out[ch * E + ei * 128:ch * E + (ei + 1) * 128, :])
                    tiles.append(t)
                return tiles

            # ---------------- bootstrap: layer-0 K/V + CA K/V ----------------
            sa_kvq = load_sa_kvq(0)
            ca_kv = load_ca_kv(0)
            G, Bt = load_ln(0)
            kfm, vsa = kv_all(sa_kvq, hch_from_sen)
            qsa = q_proj(ownfm, sa_kvq["wq"], sa_kvq["bq"])
            kca, vca = make_ca_kv(ca_kv)
            ag_out_cur = None

            for l in range(L):
                with nc.named_scope(f"L{l}"):
                    if l > 0:
                        kfm, vsa = kv_all(
                            sa_kvq, lambda ch: hch_from_ag(ag_out_cur, ch))
                    sa_o = load_sa_o(l)
                    ca_qo = load_ca_qo(l)
                    ffn_w = load_ffn_w(l)
                    if l < L - 1:
                        ca_kv_next = load_ca_kv(l + 1)

                    # ---- SA attention ----
                    attn = [attn_p.tile([128, CH], FP16, tag="attn", name="attn")
                            for _ in range(ET)]
                    with nc.named_scope("sa"):
                        attention(qsa, kfm,
                                  lambda kt, h: (vsa[kt // 4], (kt % 4) * HHW + h * HW),
                                  KT_SA, attn)

                    inter = [stm_p.tile([128, E], FP16, tag="stm", name="inter")
                             for _ in range(TT)]
                    with nc.named_scope("oln1"):
                        out_proj_ln(attn, sa_o, hid, G, Bt, inter)
                        interfm = xfm_p.tile([128, ET * CH], FP16, tag="xfm",
                                             name="interfm")
                        for t in range(TT):
                            transpose_to(interfm, inter[t], t)

                    # ---- CA ----
                    with nc.named_scope("ca"):
                        qca = q_proj(interfm, ca_qo["wq"], ca_qo["bq"])
                        if l < L - 1:
                            sa_kvq_next = load_sa_kvq(l + 1)
                        attn2 = [attn_p.tile([128, CH], FP16, tag="attn", name="attn2")
                                 for _ in range(ET)]
                        attention(qca, kca,
                                  lambda kt, h: (vca[kt // 2], (kt % 2) * HHW + h * HW),
                                  KT_CA, attn2)
                    co = [stm_p.tile([128, E], FP16, tag="stm", name="co")
                          for _ in range(TT)]
                    with nc.named_scope("oln2"):
                        out_proj_ln(attn2, ca_qo, inter, G, Bt, co)
                        cofm = xfm_p.tile([128, ET * CH], FP16, tag="xfm", name="cofm")
                        for t in range(TT):
                            transpose_to(cofm, co[t], t)

                    # ---- FFN (h1/gelu/h2 interleaved, distance 2) ----
                    with nc.named_scope("ffn"):
                        if l == L - 1:
                            hidn = [out32_p.tile([128, E], FP32, tag="out32",
                                                 name="out32") for _ in range(TT)]
                        else:
                            hidn = [stm_p.tile([128, E], FP16, tag="stm", name="hidn")
                                    for _ in range(TT)]
                        h2ps = [ps_p.tile([128, E], FP32, tag="ps", name="ps")
                                for _ in range(TT)]
                        gel = {}

                        def h2_emit(ft):
                            gt = gel.pop(ft)
                            for t in range(TT):
                                nc.tensor.matmul(h2ps[t][:], gt[:, t * 128:(t + 1) * 128],
                                                 ffn_w["w2"][ft // 4][:,
                                                 (ft % 4) * E:(ft % 4 + 1) * E],
                                                 start=(ft == 0), stop=False)

                        for ft in range(FT):
                            pst = ps_p.tile([128, CH], FP32, tag="ps", name="ps")
                            w1q = ffn_w["w1"][ft // 4]
                            for ei in range(ET):
                                nc.tensor.matmul(
                                    pst[:],
                                    w1q[:, ((ft % 4) * ET + ei) * 128:
                                        ((ft % 4) * ET + ei + 1) * 128],
                                    cofm[:, ei * CH:(ei + 1) * CH],
                                    start=(ei == 0), stop=(ei == ET - 1))
                            gt = gel_p.tile([128, CH], FP16, tag="gel", name="gel")
                            nc.scalar.activation(gt[:], pst[:], AF.Gelu,
                                                 bias=ffn_w["b1"][:, ft:ft + 1])
                            gel[ft] = gt
                            if ft >= 2:
                                h2_emit(ft - 2)
                        h2_emit(FT - 2)
                        h2_emit(FT - 1)
                        for t in range(TT):
                            nc.tensor.matmul(h2ps[t][:], onest[:], ffn_w["rb2"][:],
                                             start=False, stop=True)
                            xres = sc_p.tile([128, E], FP32, tag="xres", name="xres")
                            nc.vector.tensor_add(xres[:], h2ps[t][:], co[t][:])
                            ln_norm(xres, G, Bt, hidn[t])
                            if l == L - 1:
                                nc.sync.dma_start(out_d[t * 128:(t + 1) * 128, :],
                                                  hidn[t][:])

                    # ---- boundary: AllGather hidden; CA K/V + next Q fill it ----
                    if l < L - 1:
                        with nc.named_scope("bnd"):
                            ownfm_n = xfm_p.tile([128, ET * CH], FP16, tag="xfm",
                                                 name="ownfm_n")
                            for t in range(TT):
                                transpose_to(ownfm_n, hidn[t], t)
                            ag_in = dram_p.tile([E, CH], FP16, tag="agin", name="agin")
                            for e in range(ET):
                                nc.scalar.dma_start(
                                    ag_in[e * 128:(e + 1) * 128, :],
                                    ownfm_n[:, e * CH:(e + 1) * CH])
                            ag_out_cur = dram_p.tile([4 * E, CH], FP16, tag="agout",
                                                     name="agout")
                            nc.gpsimd.collective_compute(
                                "AllGather", OP.bypass, replica_groups=GROUPS,
                                ins=[ag_in.opt()], outs=[ag_out_cur.opt()])
                            kca, vca = make_ca_kv(ca_kv_next)
                            qsa = q_proj(ownfm_n, sa_kvq_next["wq"], sa_kvq_next["bq"])
                            Gn, Btn = load_ln(l + 1)
                        sa_kvq, ca_kv, G, Bt = sa_kvq_next, ca_kv_next, Gn, Btn
                        hid = hidn

    nc.compile()
    return nc


def _pack_ee(w):
    """[L,E,E] -> [L,128, ET*ET*128] slab: cols (ei,e,c), lhsT tile (ei,e)."""
    return np.ascontiguousarray(
        w.reshape(L, ET, 128, ET, 128).transpose(0, 2, 1, 3, 4)
        .reshape(L, 128, ET * ET * 128).astype(np.float16))


def _fm_pack(x_fm):
    """[E, T] -> [128, ET*T] (cols (e,t))."""
    t = x_fm.shape[1]
    return np.ascontiguousarray(
        x_fm.reshape(ET, 128, t).transpose(1, 0, 2).reshape(128, ET * t)
        .astype(np.float16))


def _prep_inputs(sen, know, sa_qkv_w, sa_qkv_b, sa_out_w, sa_out_b,
                 ca_qkv_w, ca_qkv_b, ca_out_w, ca_out_b,
                 ff_w1, ff_b1, ff_w2, ff_b2, ln_g, ln_b):
    f16, f32 = np.float16, np.float32

    def padv(w, b):  # [L,E,E],[L,E] -> [L,128,ET*HHW], [L,1,HHW]
        wp = np.zeros((L, E, H, HW), f32)
        wp[:, :, :, :D] = w.reshape(L, E, H, D)
        bp = np.zeros((L, H, HW), f32)
        bp[:, :, :D] = b.reshape(L, H, D)
        bp[:, :, D] = 1.0
        wsl = wp.reshape(L, ET, 128, H * HW).transpose(0, 2, 1, 3).reshape(
            L, 128, ET * HHW)
        return (np.ascontiguousarray(wsl.astype(f16)),
                np.ascontiguousarray(bp.reshape(L, 1, HHW).astype(f16)))

    wv_sa_p, rbv_sa_h = padv(sa_qkv_w[:, 2], sa_qkv_b[:, 2])
    wv_ca_p, rbv_ca_h = padv(ca_qkv_w[:, 2], ca_qkv_b[:, 2])

    def pack_o(w):  # [L,E,E] -> [L,128,ET*E]
        return np.ascontiguousarray(
            w.reshape(L, ET, 128, E).transpose(0, 2, 1, 3).reshape(L, 128, ET * E)
            .astype(f16))

    # w1: [L,E,F] -> quarter-slabs [L,4,128,4*ET*128], cols (ft%4, ei, c)
    w1q = (ff_w1.reshape(L, ET, 128, 4, 4, 128)      # (ei,p,q,ftq,c)
           .transpose(0, 3, 2, 4, 1, 5)              # (L,q,p,ftq,ei,c)
           .reshape(L, 4, 128, 4 * ET * 128))
    # w2: [L,F,E] -> quarter-slabs [L,4,128,4*E], cols (ft%4, c)
    w2q = (ff_w2.reshape(L, 4, 4, 128, E)            # (q,ftq,p,c)
           .transpose(0, 1, 3, 2, 4)                 # (L,q,p,ftq,c)
           .reshape(L, 4, 128, 4 * E))

    common = {
        "ident": np.eye(128, dtype=f16),
        "ones": np.ones((1, 128), f16),
        "wq_sa": _pack_ee(sa_qkv_w[:, 0]), "wk_sa": _pack_ee(sa_qkv_w[:, 1]),
        "wv_sa": wv_sa_p, "wo_sa": pack_o(sa_out_w),
        "wq_ca": _pack_ee(ca_qkv_w[:, 0]), "wk_ca": _pack_ee(ca_qkv_w[:, 1]),
        "wv_ca": wv_ca_p, "wo_ca": pack_o(ca_out_w),
        "w1": np.ascontiguousarray(w1q.astype(f16)),
        "w2": np.ascontiguousarray(w2q.astype(f16)),
        "bq_sa": np.ascontiguousarray(
            sa_qkv_b[:, 0].reshape(L, ET, 128).transpose(0, 2, 1)),
        "bk_sa": np.ascontiguousarray(
            sa_qkv_b[:, 1].reshape(L, ET, 128).transpose(0, 2, 1)),
        "bq_ca": np.ascontiguousarray(
            ca_qkv_b[:, 0].reshape(L, ET, 128).transpose(0, 2, 1)),
        "bk_ca": np.ascontiguousarray(
            ca_qkv_b[:, 1].reshape(L, ET, 128).transpose(0, 2, 1)),
        "b1": np.ascontiguousarray(ff_b1.reshape(L, FT, 128).transpose(0, 2, 1)),
        "rbv_sa": rbv_sa_h, "rbv_ca": rbv_ca_h,
        "rbo_sa": np.ascontiguousarray(sa_out_b[:, None, :].astype(f16)),
        "rbo_ca": np.ascontiguousarray(ca_out_b[:, None, :].astype(f16)),
        "rb2": np.ascontiguousarray(ff_b2[:, None, :].astype(f16)),
        "lng": np.ascontiguousarray(ln_g[:, None, :]),
        "lnb": np.ascontiguousarray(ln_b[:, None, :]),
    }
    in_maps = []
    for core in range(NCORES):
        g, c = core // 4, core % 4
        m = dict(common)
        m["sen_fm"] = np.ascontiguousarray(sen[g].T.astype(f16))
        m["own_fm0"] = _fm_pack(sen[g, c * CH:(c + 1) * CH].T)
        m["own_tm0"] = np.ascontiguousarray(sen[g, c * CH:(c + 1) * CH].astype(f16))
        m["know_fm"] = np.ascontiguousarray(know[g].T.astype(f16))
        in_maps.append(m)
    return in_maps


def kernel(**inputs):
    inputs = {k: np.asarray(v, dtype=np.float32) for k, v in inputs.items()}
    if "nc" not in _CACHE:
        _CACHE["nc"] = _build()
    nc = _CACHE["nc"]
    in_maps = _prep_inputs(**inputs)
    res = run_bass_kernel_spmd(nc, in_maps, list(range(NCORES)))
    out = np.empty((B, S, E), np.float32)
    for core in range(NCORES):
        g, c = core // 4, core % 4
        out[g, c * CH:(c + 1) * CH] = res.results[core]["out_tm"]
    return out


# revision 13
# speedup vs baseline: 1.6886x; 1.2342x over previous
"""Trainium2 Bass kernel for a 4-layer hierarchical-attention encoder.

Sharding: 8 cores = 2 batch groups x 4 sequence chunks of 512 query tokens.
Each core runs the full layer stack for its 512 tokens; the hidden state is
AllGathered (feature-major, 512KB) at each layer boundary and every core
recomputes full-sequence self-attention K/V locally -- the dense projection
burst also keeps the PE at its top p-state.  Cross-attention K/V from the
static `know` are computed locally per layer during the AllGather window.

Attention is software-pipelined (scores for tile kt issue while PV matmuls
for kt-2 retire) so the PE does not sit behind the softmax exp; exp tiles
are split between the Activation engine (exact exp) and the Vector engine
(Schraudolph fp16 bit-trick exp; the ~3% per-prob error washes out in the
softmax normalization and LayerNorm).  Weights load as whole-layer slabs.
"""
import os
import sys

for _p in ("/root/.axon_site/_ro/trn_rl_repo", "/opt/trn_rl_repo", "/opt/pypackages",
           "/root/.axon_site/_ro/pypackages"):
    if os.path.isdir(_p) and _p not in sys.path:
        sys.path.append(_p)

import numpy as np

import concourse.bass as bass
import concourse.mybir as mybir
import concourse.tile as tile
from concourse import bacc
from concourse.bass_utils import run_bass_kernel_spmd

L, E, H, D, F = 4, 512, 8, 64, 2048
B, S, SK = 2, 2048, 1024
NCORES = 8
GROUPS = [[0, 1, 2, 3], [4, 5, 6, 7]]
CH = 512          # query tokens per core
ET = E // 128     # 4 feature tiles
TT = CH // 128    # 4 token tiles in own chunk
FT = F // 128     # 16 ffn tiles
KT_SA = S // 128  # 16 key tiles (self)
KT_CA = SK // 128  # 8 key tiles (cross)
HW = 65           # head width incl. denominator column
HHW = H * HW      # 520

FP32 = mybir.dt.float32
FP16 = mybir.dt.float16
I16 = mybir.dt.int16
AF = mybir.ActivationFunctionType
OP = mybir.AluOpType

# Schraudolph fp16 exp: e^(0.125*x) ~= bitcast_fp16(int16(round(A*x + B)))
SEXP_A = float(1024.0 / np.log(2.0)) * 0.125
SEXP_B = 15301.0
SEXP_MODE = os.environ.get("USE_SEXP", "0")  # 0|sa|ca|1

_CACHE = {}


def _build():
    nc = bacc.Bacc("TRN2", target_bir_lowering=False, debug=False, num_devices=NCORES)

    def din(name, shape, dt=FP16):
        return nc.dram_tensor(name, shape, dt, kind="ExternalInput").ap()

    sen_fm = din("sen_fm", [E, S])             # full batch-group seq, feature-major
    own_fm0 = din("own_fm0", [128, ET * CH])   # own chunk, feature-major packed
    own_tm0 = din("own_tm0", [CH, E])          # own chunk, token-major
    know_fm_d = din("know_fm", [E, SK])        # full know, feature-major
    ident_d = din("ident", [128, 128])
    ones_d = din("ones", [1, 128])

    wq_sa_d = din("wq_sa", [L, 128, ET * ET * 128])
    wk_sa_d = din("wk_sa", [L, 128, ET * ET * 128])
    wv_sa_d = din("wv_sa", [L, 128, ET * HHW])
    wo_sa_d = din("wo_sa", [L, 128, ET * E])
    wq_ca_d = din("wq_ca", [L, 128, ET * ET * 128])
    wk_ca_d = din("wk_ca", [L, 128, ET * ET * 128])
    wv_ca_d = din("wv_ca", [L, 128, ET * HHW])
    wo_ca_d = din("wo_ca", [L, 128, ET * E])
    w1_d = din("w1", [L, 4, 128, 4 * ET * 128])   # quarter-slabs, ft-major
    w2_d = din("w2", [L, 4, 128, 4 * E])          # quarter-slabs, ft-major

    bq_sa_d = din("bq_sa", [L, 128, ET], FP32)
    bk_sa_d = din("bk_sa", [L, 128, ET], FP32)
    bq_ca_d = din("bq_ca", [L, 128, ET], FP32)
    bk_ca_d = din("bk_ca", [L, 128, ET], FP32)
    b1_d = din("b1", [L, 128, FT], FP32)
    rbv_sa_d = din("rbv_sa", [L, 1, HHW])
    rbo_sa_d = din("rbo_sa", [L, 1, E])
    rbv_ca_d = din("rbv_ca", [L, 1, HHW])
    rbo_ca_d = din("rbo_ca", [L, 1, E])
    rb2_d = din("rb2", [L, 1, E])
    lng_d = din("lng", [L, 1, E], FP32)
    lnb_d = din("lnb", [L, 1, E], FP32)

    out_d = nc.dram_tensor("out_tm", [CH, E], FP32, kind="ExternalOutput").ap()

    with tile.TileContext(nc) as tc:
        from contextlib import ExitStack
        with ExitStack() as ctx:
            ep = ctx.enter_context
            const_p = ep(tc.tile_pool(name="const", bufs=1))
            know_p = ep(tc.tile_pool(name="know", bufs=4))
            hch_p = ep(tc.tile_pool(name="hch", bufs=8))      # [128,512] hidden fm
            kfm_p = ep(tc.tile_pool(name="kfm", bufs=4))      # [128,2048] SA K fp16
            vsa_p = ep(tc.tile_pool(name="vsa", bufs=4))      # [128,2080] SA V fp16
            kca_p = ep(tc.tile_pool(name="kca", bufs=4))      # [128,1024] CA K fp16
            vca_p = ep(tc.tile_pool(name="vca", bufs=4))      # [128,1040] CA V fp16
            xfm_p = ep(tc.tile_pool(name="xfm", bufs=4))      # [128,2048] fm acts
            qfm_p = ep(tc.tile_pool(name="qfm", bufs=4))
            attn_p = ep(tc.tile_pool(name="attn", bufs=8))
            stm_p = ep(tc.tile_pool(name="stm", bufs=10))     # hid/inter/co TM fp16
            out32_p = ep(tc.tile_pool(name="out32", bufs=4))
            pt_p = ep(tc.tile_pool(name="pt", bufs=4))        # exp(scores) rings
            gel_p = ep(tc.tile_pool(name="gel", bufs=3))
            wsl_p = ep(tc.tile_pool(name="wsl", bufs=1))      # weight slabs
            row_p = ep(tc.tile_pool(name="row", bufs=4))      # [1,<=520] rows
            gb_p = ep(tc.tile_pool(name="gb", bufs=1))        # LN G/B broadcast fp32
            sc_p = ep(tc.tile_pool(name="sc", bufs=2))        # fp32 scratch
            s1_p = ep(tc.tile_pool(name="s1", bufs=1))        # [<=4,512] rows
            st_p = ep(tc.tile_pool(name="st", bufs=8))        # small stats
            ps_p = ep(tc.tile_pool(name="ps", bufs=6, space="PSUM"))
            dram_p = ep(tc.tile_pool(name="dram", bufs=2, space="DRAM"))

            identt = const_p.tile([128, 128], FP16, tag="ident", name="ident")
            nc.sync.dma_start(identt[:], ident_d[:])
            onest = const_p.tile([1, 128], FP16, tag="ones", name="ones")
            nc.sync.dma_start(onest[:], ones_d[:])
            knowfm = []
            for e in range(ET):
                t = know_p.tile([128, SK], FP16, tag="know", name="know")
                nc.sync.dma_start(t[:], know_fm_d[e * 128:(e + 1) * 128, :])
                knowfm.append(t)
            ownfm = xfm_p.tile([128, ET * CH], FP16, tag="xfm", name="ownfm")
            nc.sync.dma_start(ownfm[:], own_fm0[:])
            hid = []
            for t in range(TT):
                h = stm_p.tile([128, E], FP16, tag="stm", name="hid")
                nc.sync.dma_start(h[:], own_tm0[t * 128:(t + 1) * 128, :])
                hid.append(h)

            # ---------------- weight loaders (single-buffered slabs) ----
            def slab(dram, l, cols, tg, bufs=1):
                t = wsl_p.tile([128, cols], FP16, tag=tg, name=tg, bufs=bufs)
                nc.sync.dma_start(t[:], dram[l])
                return t

            def srow(dram, l, cols, tg="row"):
                t = row_p.tile([1, cols], FP16, tag=tg, name=tg, bufs=8)
                nc.sync.dma_start(t[:], dram[l])
                return t

            def sbias(dram, l, cols, tg="bias"):
                t = st_p.tile([128, cols], FP32, tag=tg, name=tg, bufs=8)
                nc.sync.dma_start(t[:], dram[l])
                return t

            def load_sa_kvq(l):
                return dict(
                    wq=slab(wq_sa_d, l, ET * ET * 128, "wq_sa"),
                    wk=slab(wk_sa_d, l, ET * ET * 128, "wk_sa"),
                    wv=slab(wv_sa_d, l, ET * HHW, "wv_sa"),
                    bq=sbias(bq_sa_d, l, ET), bk=sbias(bk_sa_d, l, ET),
                    rbv=srow(rbv_sa_d, l, HHW))

            def load_sa_o(l):
                return dict(wo=slab(wo_sa_d, l, ET * E, "wo_sa"),
                            rbo=srow(rbo_sa_d, l, E))

            def load_ca_kv(l):
                return dict(
                    wk=slab(wk_ca_d, l, ET * ET * 128, "wk_ca"),
                    wv=slab(wv_ca_d, l, ET * HHW, "wv_ca"),
                    bk=sbias(bk_ca_d, l, ET), rbv=srow(rbv_ca_d, l, HHW))

            def load_ca_qo(l):
                return dict(
                    wq=slab(wq_ca_d, l, ET * ET * 128, "wq_ca"),
                    wo=slab(wo_ca_d, l, ET * E, "wo_ca"),
                    bq=sbias(bq_ca_d, l, ET), rbo=srow(rbo_ca_d, l, E))

            def load_ffn_w(l):
                return dict(
                    w1=[slab(w1_d[l], q, 4 * ET * 128, "w1q", bufs=2)
                        for q in range(4)],
                    w2=[slab(w2_d[l], q, 4 * E, "w2q", bufs=2) for q in range(4)],
                    b1=sbias(b1_d, l, FT), rb2=srow(rb2_d, l, E))

            def load_ln(l):
                lr = s1_p.tile([1, E], FP32, tag="lnrow", name="lnrow")
                nc.sync.dma_start(lr[:], lng_d[l])
                G = gb_p.tile([128, E], FP32, tag="G", name="G")
                nc.gpsimd.partition_broadcast(G[:], lr[:])
                lr2 = s1_p.tile([1, E], FP32, tag="B", name="lnrow2")
                nc.sync.dma_start(lr2[:], lnb_d[l])
                Bt = gb_p.tile([128, E], FP32, tag="Bb", name="Bb")
                nc.gpsimd.partition_broadcast(Bt[:], lr2[:])
                return G, Bt

            # ---------------- compute helpers ----------------
            def ln_norm(xres, G, Bt, out):
                """out = G*(xres-mean)/(sqrt(bessel_var)+eps) + Bt."""
                stt = st_p.tile([128, 6], FP32, tag="bnst", name="bnst")
                nc.vector.bn_stats(out=stt[:], in_=xres[:])
                mv = st_p.tile([128, 2], FP32, tag="bnmv", name="bnmv")
                nc.vector.bn_aggr(out=mv[:], in_=stt[:])
                sd = st_p.tile([128, 1], FP32, tag="sd", name="sd")
                nc.scalar.activation(sd[:], mv[:, 1:2], AF.Sqrt, scale=float(E) / (E - 1))
                nc.vector.tensor_scalar_add(sd[:], sd[:], 1e-6)
                inv = st_p.tile([128, 1], FP32, tag="inv", name="inv")
                nc.vector.reciprocal_approx_fast(inv[:], sd[:])
                minv = st_p.tile([128, 1], FP32, tag="minv", name="minv")
                nc.vector.tensor_mul(minv[:], mv[:, 0:1], inv[:])
                tmp = sc_p.tile([128, E], FP32, tag="lntmp", name="lntmp")
                nc.vector.tensor_scalar(tmp[:], in0=xres[:], scalar1=inv[:],
                                        scalar2=minv[:], op0=OP.mult, op1=OP.subtract)
                nc.vector.tensor_mul(tmp[:], tmp[:], G[:])
                nc.vector.tensor_add(out[:], tmp[:], Bt[:])

            def transpose_to(dst, src_tile, t):
                """src [128tok, E] TM tile t -> dst[:, e*CH + t*128 ...] (fm)."""
                for e in range(ET):
                    tp = ps_p.tile([128, 128], FP16, tag="ps", name="ps")
                    nc.tensor.transpose(tp[:], src_tile[:, e * 128:(e + 1) * 128],
                                        identt[:])
                    nc.vector.tensor_copy(dst[:, e * CH + t * 128:e * CH + (t + 1) * 128],
                                          tp[:])

            def q_proj(src_fm, wq, bq):
                qs = []
                for e in range(ET):
                    pst = ps_p.tile([128, CH], FP32, tag="ps", name="ps")
                    for ei in range(ET):
                        nc.tensor.matmul(
                            pst[:], wq[:, (ei * ET + e) * 128:(ei * ET + e + 1) * 128],
                            src_fm[:, ei * CH:(ei + 1) * CH],
                            start=(ei == 0), stop=(ei == ET - 1))
                    qt = qfm_p.tile([128, CH], FP16, tag="qfm", name="qfm")
                    nc.vector.tensor_scalar_add(qt[:], pst[:], bq[:, e:e + 1])
                    qs.append(qt)
                return qs

            def kv_all(w, src_fn):
                """Full-sequence SA K/V from per-chunk hidden fm tiles.

                src_fn(ch) -> list of 4 [128,512] fm tiles for chunk ch."""
                kfm = [kfm_p.tile([128, S], FP16, tag="kfm", name="kfm")
                       for _ in range(ET)]
                vsa = [vsa_p.tile([128, 4 * HHW], FP16, tag="vsa", name="vsa")
                       for _ in range(4)]
                for ch in range(4):
                    hch = src_fn(ch)
                    for e in range(ET):
                        pst = ps_p.tile([128, CH], FP32, tag="ps", name="ps")
                        for ei in range(ET):
                            nc.tensor.matmul(
                                pst[:],
                                w["wk"][:, (ei * ET + e) * 128:(ei * ET + e + 1) * 128],
                                hch[ei][:], start=(ei == 0), stop=(ei == ET - 1))
                        nc.vector.tensor_scalar_add(
                            kfm[e][:, ch * CH:(ch + 1) * CH], pst[:], w["bk"][:, e:e + 1])
                    for lt in range(TT):
                        for half in range(2):
                            cs = half * (HHW // 2)
                            pst = ps_p.tile([128, HHW // 2], FP32, tag="ps", name="ps")
                            for ei in range(ET):
                                nc.tensor.matmul(
                                    pst[:], hch[ei][:, lt * 128:(lt + 1) * 128],
                                    w["wv"][:, ei * HHW + cs:ei * HHW + cs + HHW // 2],
                                    start=(ei == 0), stop=False)
                            nc.tensor.matmul(pst[:], onest[:],
                                             w["rbv"][:, cs:cs + HHW // 2],
                                             start=False, stop=True)
                            nc.scalar.activation(
                                vsa[ch][:, lt * HHW + cs:lt * HHW + cs + HHW // 2],
                                pst[:], AF.Copy)
                return kfm, vsa

            def make_ca_kv(w):
                """Full CA K/V from resident know (collective-window filler)."""
                kca = [kca_p.tile([128, SK], FP16, tag="kca", name="kca")
                       for _ in range(ET)]
                vca = [vca_p.tile([128, 2 * HHW], FP16, tag="vca", name="vca")
                       for _ in range(4)]
                for e in range(ET):
                    for cc in range(2):
                        pst = ps_p.tile([128, CH], FP32, tag="ps", name="ps")
                        for ei in range(ET):
                            nc.tensor.matmul(
                                pst[:],
                                w["wk"][:, (ei * ET + e) * 128:(ei * ET + e + 1) * 128],
                                knowfm[ei][:, cc * CH:(cc + 1) * CH],
                                start=(ei == 0), stop=(ei == ET - 1))
                        nc.vector.tensor_scalar_add(
                            kca[e][:, cc * CH:(cc + 1) * CH], pst[:], w["bk"][:, e:e + 1])
                for kt in range(KT_CA):
                    for half in range(2):
                        cs = half * (HHW // 2)
                        pst = ps_p.tile([128, HHW // 2], FP32, tag="ps", name="ps")
                        for ei in range(ET):
                            nc.tensor.matmul(
                                pst[:], knowfm[ei][:, kt * 128:(kt + 1) * 128],
                                w["wv"][:, ei * HHW + cs:ei * HHW + cs + HHW // 2],
                                start=(ei == 0), stop=False)
                        nc.tensor.matmul(pst[:], onest[:], w["rbv"][:, cs:cs + HHW // 2],
                                         start=False, stop=True)
                        nc.scalar.activation(
                            vca[kt // 2][:, (kt % 2) * HHW + cs:(kt % 2) * HHW + cs + HHW // 2],
                            pst[:], AF.Copy)
                return kca, vca

            def attention(qfm, kfm, vp_at, nkt, attn_tiles, sexp=False):
                PD = 2  # exp pipeline distance in kt tiles
                for hp in range(ET):
                    attps = [ps_p.tile([HW, CH], FP32, tag="attps", name="attps",
                                       bufs=2)
                             for _ in range(2)]
                    pts = {}

                    def scores(kt):
                        for j in (0, 1):
                            spt = ps_p.tile([128, CH], FP32, tag="ps", name="spt")
                            nc.tensor.matmul(
                                spt[:],
                                kfm[hp][j * 64:(j + 1) * 64, kt * 128:(kt + 1) * 128],
                                qfm[hp][j * 64:(j + 1) * 64, :], start=True, stop=True)
                            if j == 0 or not sexp:
                                pt = pt_p.tile([128, CH], FP16, tag="pte", name="pte")
                                nc.scalar.activation(pt[:], spt[:], AF.Exp, scale=0.125)
                                pts[kt, j] = pt[:]
                            else:
                                pti = pt_p.tile([128, CH], FP16, tag="ptv", name="ptv")
                                nc.vector.tensor_scalar(pti[:].bitcast(I16), in0=spt[:],
                                                        scalar1=SEXP_A, scalar2=SEXP_B,
                                                        op0=OP.mult, op1=OP.add)
                                pts[kt, j] = pti[:]

                    def pv(kt):
                        for j in (0, 1):
                            h = hp * 2 + j
                            vtile, col0 = vp_at(kt, h)
                            nc.tensor.matmul(attps[j][:], vtile[:, col0:col0 + HW],
                                             pts.pop((kt, j)),
                                             start=(kt == 0), stop=(kt == nkt - 1))

                    for kt in range(nkt):
                        scores(kt)
                        if kt >= PD:
                            pv(kt - PD)
                    for kt in range(nkt - PD, nkt):
                        pv(kt)
                    for j in (0, 1):
                        den = s1_p.tile([1, CH], FP32, tag="den", name="den")
                        nc.scalar.activation(den[:], attps[j][64:65, :], AF.Copy)
                        rec = s1_p.tile([1, CH], FP32, tag="rec", name="rec")
                        nc.vector.reciprocal_approx_fast(rec[:], den[:])
                        rb = sc_p.tile([64, CH], FP32, tag="rb", name="rb")
                        nc.gpsimd.partition_broadcast(rb[:], rec[:])
                        nc.vector.tensor_mul(attn_tiles[hp][j * 64:(j + 1) * 64, :],
                                             attps[j][0:64, :], rb[:])

            def out_proj_ln(attn_tiles, w, res, G, Bt, out_tiles):
                for t in range(TT):
                    pst = ps_p.tile([128, E], FP32, tag="ps", name="ps")
                    for ei in range(ET):
                        nc.tensor.matmul(pst[:], attn_tiles[ei][:, t * 128:(t + 1) * 128],
                                         w["wo"][:, ei * E:(ei + 1) * E],
                                         start=(ei == 0), stop=False)
                    nc.tensor.matmul(pst[:], onest[:], w["rbo"][:], start=False, stop=True)
                    xres = sc_p.tile([128, E], FP32, tag="xres", name="xres")
                    nc.vector.tensor_add(xres[:], pst[:], res[t][:])
                    ln_norm(xres, G, Bt, out_tiles[t])

            def hch_from_sen(ch):
                tiles = []
                for ei in range(ET):
                    t = hch_p.tile([128, CH], FP16, tag="hch", name="hch")
                    nc.sync.dma_start(t[:], sen_fm[ei * 128:(ei + 1) * 128,
                                                   ch * CH:(ch + 1) * CH])
                    tiles.append(t)
                return tiles

            def hch_from_ag(ag_out, ch):
                tiles = []
                for ei in range(ET):
                    t = hch_p.tile([128, CH], FP16, tag="hch", name="hch")
                    nc.sync.dma_start(
                        t[:], ag_out[ch * E + ei * 128:ch * E + (ei + 1) * 128, :])
                    tiles.append(t)
                return tiles

            # ---------------- bootstrap: layer-0 K/V + CA K/V ----------------
            sa_kvq = load_sa_kvq(0)
            ca_kv = load_ca_kv(0)
            G, Bt = load_ln(0)
            kfm, vsa = kv_all(sa_kvq, hch_from_sen)
            qsa = q_proj(ownfm, sa_kvq["wq"], sa_kvq["bq"])
            kca, vca = make_ca_kv(ca_kv)
            ag_out_cur = None

            for l in range(L):
                with nc.named_scope(f"L{l}"):
                    if l > 0:
                        kfm, vsa = kv_all(
                            sa_kvq, lambda ch: hch_from_ag(ag_out_cur, ch))
                    sa_o = load_sa_o(l)
                    ca_qo = load_ca_qo(l)
                    ffn_w = load_ffn_w(l)
                    if l < L - 1:
                        ca_kv_next = load_ca_kv(l + 1)

                    # ---- SA attention ----
                    attn = [attn_p.tile([128, CH], FP16, tag="attn", name="attn")
                            for _ in range(ET)]
                    with nc.named_scope("sa"):
                        attention(qsa, kfm,
                                  lambda kt, h: (vsa[kt // 4], (kt % 4) * HHW + h * HW),
                                  KT_SA, attn, sexp=SEXP_MODE in ("1", "sa"))

                    inter = [stm_p.tile([128, E], FP16, tag="stm", name="inter")
                             for _ in range(TT)]
                    with nc.named_scope("oln1"):
                        out_proj_ln(attn, sa_o, hid, G, Bt, inter)
                        interfm = xfm_p.tile([128, ET * CH], FP16, tag="xfm",
                                             name="interfm")
                        for t in range(TT):
                            transpose_to(interfm, inter[t], t)

                    # ---- CA ----
                    with nc.named_scope("ca"):
                        qca = q_proj(interfm, ca_qo["wq"], ca_qo["bq"])
                        if l < L - 1:
                            sa_kvq_next = load_sa_kvq(l + 1)
                        attn2 = [attn_p.tile([128, CH], FP16, tag="attn", name="attn2")
                                 for _ in range(ET)]
                        attention(qca, kca,
                                  lambda kt, h: (vca[kt // 2], (kt % 2) * HHW + h * HW),
                                  KT_CA, attn2, sexp=SEXP_MODE in ("1", "ca"))
                    co = [stm_p.tile([128, E], FP16, tag="stm", name="co")
                          for _ in range(TT)]
                    with nc.named_scope("oln2"):
                        out_proj_ln(attn2, ca_qo, inter, G, Bt, co)
                        cofm = xfm_p.tile([128, ET * CH], FP16, tag="xfm", name="cofm")
                        for t in range(TT):
                            transpose_to(cofm, co[t], t)

                    # ---- FFN (h1/gelu/h2 interleaved, distance 2) ----
                    with nc.named_scope("ffn"):
                        if l == L - 1:
                            hidn = [out32_p.tile([128, E], FP32, tag="out32",
                                                 name="out32") for _ in range(TT)]
                        else:
                            hidn = [stm_p.tile([128, E], FP16, tag="stm", name="hidn")
                                    for _ in range(TT)]
                        h2ps = [ps_p.tile([128, E], FP32, tag="ps", name="ps")
                                for _ in range(TT)]
                        gel = {}

                        def h2_emit(ft):
                            gt = gel.pop(ft)
                            for t in range(TT):
                                nc.tensor.matmul(h2ps[t][:], gt[:, t * 128:(t + 1) * 128],
                                                 ffn_w["w2"][ft // 4][:,
                                                 (ft % 4) * E:(ft % 4 + 1) * E],
                                                 start=(ft == 0), stop=False)

                        for ft in range(FT):
                            pst = ps_p.tile([128, CH], FP32, tag="ps", name="ps")
                            w1q = ffn_w["w1"][ft // 4]
                            for ei in range(ET):
                                nc.tensor.matmul(
                                    pst[:],
                                    w1q[:, ((ft % 4) * ET + ei) * 128:
                                        ((ft % 4) * ET + ei + 1) * 128],
                                    cofm[:, ei * CH:(ei + 1) * CH],
                                    start=(ei == 0), stop=(ei == ET - 1))
                            gt = gel_p.tile([128, CH], FP16, tag="gel", name="gel")
                            nc.scalar.activation(gt[:], pst[:], AF.Gelu,
                                                 bias=ffn_w["b1"][:, ft:ft + 1])
                            gel[ft] = gt
                            if ft >= 2:
                                h2_emit(ft - 2)
                        h2_emit(FT - 2)
                        h2_emit(FT - 1)
                        for t in range(TT):
                            nc.tensor.matmul(h2ps[t][:], onest[:], ffn_w["rb2"][:],
                                             start=False, stop=True)
                            xres = sc_p.tile([128, E], FP32, tag="xres", name="xres")
                            nc.vector.tensor_add(xres[:], h2ps[t][:], co[t][:])
                            ln_norm(xres, G, Bt, hidn[t])
                            if l == L - 1:
                                nc.sync.dma_start(out_d[t * 128:(t + 1) * 128, :],
                                                  hidn[t][:])

                    # ---- boundary: AllGather hidden; CA K/V + next Q fill it ----
                    if l < L - 1:
                        with nc.named_scope("bnd"):
                            ownfm_n = xfm_p.tile([128, ET * CH], FP16, tag="xfm",
                                                 name="ownfm_n")
                            for t in range(TT):
                                transpose_to(ownfm_n, hidn[t], t)
                            ag_in = dram_p.tile([E, CH], FP16, tag="agin", name="agin")
                            for e in range(ET):
                                nc.scalar.dma_start(
                                    ag_in[e * 128:(e + 1) * 128, :],
                                    ownfm_n[:, e * CH:(e + 1) * CH])
                            ag_out_cur = dram_p.tile([4 * E, CH], FP16, tag="agout",
                                                     name="agout")
                            nc.gpsimd.collective_compute(
                                "AllGather", OP.bypass, replica_groups=GROUPS,
                                ins=[ag_in.opt()], outs=[ag_out_cur.opt()])
                            kca, vca = make_ca_kv(ca_kv_next)
                            qsa = q_proj(ownfm_n, sa_kvq_next["wq"], sa_kvq_next["bq"])
                            Gn, Btn = load_ln(l + 1)
                        sa_kvq, ca_kv, G, Bt = sa_kvq_next, ca_kv_next, Gn, Btn
                        hid = hidn

    nc.compile()
    return nc


def _pack_ee(w):
    """[L,E,E] -> [L,128, ET*ET*128] slab: cols (ei,e,c), lhsT tile (ei,e)."""
    return np.ascontiguousarray(
        w.reshape(L, ET, 128, ET, 128).transpose(0, 2, 1, 3, 4)
        .reshape(L, 128, ET * ET * 128).astype(np.float16))


def _fm_pack(x_fm):
    """[E, T] -> [128, ET*T] (cols (e,t))."""
    t = x_fm.shape[1]
    return np.ascontiguousarray(
        x_fm.reshape(ET, 128, t).transpose(1, 0, 2).reshape(128, ET * t)
        .astype(np.float16))


def _prep_inputs(sen, know, sa_qkv_w, sa_qkv_b, sa_out_w, sa_out_b,
                 ca_qkv_w, ca_qkv_b, ca_out_w, ca_out_b,
                 ff_w1, ff_b1, ff_w2, ff_b2, ln_g, ln_b):
    f16, f32 = np.float16, np.float32

    def padv(w, b):  # [L,E,E],[L,E] -> [L,128,ET*HHW], [L,1,HHW]
        wp = np.zeros((L, E, H, HW), f32)
        wp[:, :, :, :D] = w.reshape(L, E, H, D)
        bp = np.zeros((L, H, HW), f32)
        bp[:, :, :D] = b.reshape(L, H, D)
        bp[:, :, D] = 1.0
        wsl = wp.reshape(L, ET, 128, H * HW).transpose(0, 2, 1, 3).reshape(
            L, 128, ET * HHW)
        return (np.ascontiguousarray(wsl.astype(f16)),
                np.ascontiguousarray(bp.reshape(L, 1, HHW).astype(f16)))

    wv_sa_p, rbv_sa_h = padv(sa_qkv_w[:, 2], sa_qkv_b[:, 2])
    wv_ca_p, rbv_ca_h = padv(ca_qkv_w[:, 2], ca_qkv_b[:, 2])

    def pack_o(w):  # [L,E,E] -> [L,128,ET*E]
        return np.ascontiguousarray(
            w.reshape(L, ET, 128, E).transpose(0, 2, 1, 3).reshape(L, 128, ET * E)
            .astype(f16))

    # w1: [L,E,F] -> quarter-slabs [L,4,128,4*ET*128], cols (ft%4, ei, c)
    w1q = (ff_w1.reshape(L, ET, 128, 4, 4, 128)      # (ei,p,q,ftq,c)
           .transpose(0, 3, 2, 4, 1, 5)              # (L,q,p,ftq,ei,c)
           .reshape(L, 4, 128, 4 * ET * 128))
    # w2: [L,F,E] -> quarter-slabs [L,4,128,4*E], cols (ft%4, c)
    w2q = (ff_w2.reshape(L, 4, 4, 128, E)            # (q,ftq,p,c)
           .transpose(0, 1, 3, 2, 4)                 # (L,q,p,ftq,c)
           .reshape(L, 4, 128, 4 * E))

    common = {
        "ident": np.eye(128, dtype=f16),
        "ones": np.ones((1, 128), f16),
        "wq_sa": _pack_ee(sa_qkv_w[:, 0]), "wk_sa": _pack_ee(sa_qkv_w[:, 1]),
        "wv_sa": wv_sa_p, "wo_sa": pack_o(sa_out_w),
        "wq_ca": _pack_ee(ca_qkv_w[:, 0]), "wk_ca": _pack_ee(ca_qkv_w[:, 1]),
        "wv_ca": wv_ca_p, "wo_ca": pack_o(ca_out_w),
        "w1": np.ascontiguousarray(w1q.astype(f16)),
        "w2": np.ascontiguousarray(w2q.astype(f16)),
        "bq_sa": np.ascontiguousarray(
            sa_qkv_b[:, 0].reshape(L, ET, 128).transpose(0, 2, 1)),
        "bk_sa": np.ascontiguousarray(
            sa_qkv_b[:, 1].reshape(L, ET, 128).transpose(0, 2, 1)),
        "bq_ca": np.ascontiguousarray(
            ca_qkv_b[:, 0].reshape(L, ET, 128).transpose(0, 2, 1)),
        "bk_ca": np.ascontiguousarray(
            ca_qkv_b[:, 1].reshape(L, ET, 128).transpose(0, 2, 1)),
        "b1": np.ascontiguousarray(ff_b1.reshape(L, FT, 128).transpose(0, 2, 1)),
        "rbv_sa": rbv_sa_h, "rbv_ca": rbv_ca_h,
        "rbo_sa": np.ascontiguousarray(sa_out_b[:, None, :].astype(f16)),
        "rbo_ca": np.ascontiguousarray(ca_out_b[:, None, :].astype(f16)),
        "rb2": np.ascontiguousarray(ff_b2[:, None, :].astype(f16)),
        "lng": np.ascontiguousarray(ln_g[:, None, :]),
        "lnb": np.ascontiguousarray(ln_b[:, None, :]),
    }
    in_maps = []
    for core in range(NCORES):
        g, c = core // 4, core % 4
        m = dict(common)
        m["sen_fm"] = np.ascontiguousarray(sen[g].T.astype(f16))
        m["own_fm0"] = _fm_pack(sen[g, c * CH:(c + 1) * CH].T)
        m["own_tm0"] = np.ascontiguousarray(sen[g, c * CH:(c + 1) * CH].astype(f16))
        m["know_fm"] = np.ascontiguousarray(know[g].T.astype(f16))
        in_maps.append(m)
    return in_maps


def kernel(**inputs):
    inputs = {k: np.asarray(v, dtype=np.float32) for k, v in inputs.items()}
    if "nc" not in _CACHE:
        _CACHE["nc"] = _build()
    nc = _CACHE["nc"]
    in_maps = _prep_inputs(**inputs)
    res = run_bass_kernel_spmd(nc, in_maps, list(range(NCORES)))
    out = np.empty((B, S, E), np.float32)
    for core in range(NCORES):
        g, c = core // 4, core % 4
        out[g, c * CH:(c + 1) * CH] = res.results[core]["out_tm"]
    return out
